# revision 1
# baseline (speedup 1.0000x reference)
"""CondRotConv2d Trainium2 kernel.

Sharding: pure batch DP, 2 images per core across 8 cores.

Per-core layout: 4 partition groups g = 2*b_local + half; group g holds the 32
channels of one half-image (128 rows) zero/halo-padded to [130, 258] in SBUF.
Conv = 9 shifted-AP matmuls (K=32 cin, M=32 cout, N=512 px) accumulated in
PSUM; 16 concurrent 32x32 PE subarrays via tile_position (row group = data
group g, col group j = output tile slot); each subarray accumulates its own
(partition-slot j, bank g) PSUM region.

Gating path: per-partition sums -> one K=128 matmul against a host-built
block matrix (folds group-combine + both FC layers + /HW) -> sigmoid ->
DRAM-bounce broadcast onto (e,j) partitions -> rotation matrix built with
DVE/ACT elementwise ops -> 64 small K=72 matmuls produce per-image combined
conv kernels -> replicated to all partition groups via DRAM.
"""

import numpy as np

import concourse.bass as bass
import concourse.tile as tile
import concourse.mybir as mybir
from concourse import bacc
from concourse.bass_utils import run_bass_kernel_spmd

dt = mybir.dt
AF = mybir.ActivationFunctionType

B, CIN, COUT, H, W, E = 16, 32, 32, 256, 256, 8
NCORES = 8
BL = B // NCORES          # local images per core = 2
NG = 2 * BL               # partition groups = 4
HH = H // 2               # half-image rows = 128
PW = W + 2                # padded width = 258
PR = HH + 2               # padded rows = 130
ROUNDS = 16               # supertile rounds (8 image rows per round per group)

_rc = np.stack(np.meshgrid(np.arange(3) - 1, np.arange(3) - 1,
                           indexing="ij"), -1).reshape(9, 2).astype(np.float32)
GRID_R, GRID_C = _rc[:, 0].copy(), _rc[:, 1].copy()


def build_module(wdt=dt.float32, loop=False):
    nc = bacc.Bacc(None, target_bir_lowering=False, debug=False)

    x_d = nc.dram_tensor("x", [BL, CIN, H, W], dt.float32, kind="ExternalInput")
    wf_d = nc.dram_tensor("wf_t", [72, 1024], dt.float32, kind="ExternalInput")
    fc2_d = nc.dram_tensor("fc2", [128, 32], dt.float32, kind="ExternalInput")
    fcb2_d = nc.dram_tensor("fcb2", [32, 1], dt.float32, kind="ExternalInput")
    grt_d = nc.dram_tensor("grt", [72, 18], dt.float32, kind="ExternalInput")
    gct_d = nc.dram_tensor("gct", [72, 18], dt.float32, kind="ExternalInput")
    grj_d = nc.dram_tensor("grj", [72, 1], dt.float32, kind="ExternalInput")
    gcj_d = nc.dram_tensor("gcj", [72, 1], dt.float32, kind="ExternalInput")
    out_d = nc.dram_tensor("out", [BL, COUT, H, W], dt.float32, kind="ExternalOutput")
    if loop:
        nit_d = nc.dram_tensor("niters", [1, 1], dt.int32, kind="ExternalInput")

    with tile.TileContext(nc) as tc:
        with (
            tc.tile_pool(name="xp", bufs=1) as xp,
            tc.tile_pool(name="cst", bufs=1) as cst,
            tc.tile_pool(name="wk", bufs=2) as wk,
            tc.tile_pool(name="st", bufs=3) as stp,
            tc.tile_pool(name="dr", bufs=2, space="DRAM") as drp,
            tc.tile_pool(name="ps", bufs=1, space="PSUM") as ps,
        ):
            # ---- constants ----
            wf_sb = cst.tile([72, 1024], dt.float32)
            nc.sync.dma_start(out=wf_sb[:], in_=wf_d.ap())
            fc2_sb = cst.tile([128, 32], dt.float32)
            nc.sync.dma_start(out=fc2_sb[:], in_=fc2_d.ap())
            fcb2_sb = cst.tile([32, 1], dt.float32)
            nc.sync.dma_start(out=fcb2_sb[:], in_=fcb2_d.ap())
            grt_sb = cst.tile([72, 18], dt.float32)
            nc.sync.dma_start(out=grt_sb[:], in_=grt_d.ap())
            gct_sb = cst.tile([72, 18], dt.float32)
            nc.sync.dma_start(out=gct_sb[:], in_=gct_d.ap())
            grj_sb = cst.tile([72, 1], dt.float32)
            nc.sync.dma_start(out=grj_sb[:], in_=grj_d.ap())
            gcj_sb = cst.tile([72, 1], dt.float32)
            nc.sync.dma_start(out=gcj_sb[:], in_=gcj_d.ap())
            halfpi_sb = cst.tile([72, 1], dt.float32)
            nc.gpsimd.memset(halfpi_sb[:], float(np.pi / 2))

            if loop:
                nit_sb = cst.tile([1, 1], dt.int32)
                nc.sync.dma_start(out=nit_sb[:], in_=nit_d.ap())
                n_rv = nc.values_load(nit_sb[0:1, 0:1], min_val=0, max_val=100000,
                                      skip_runtime_bounds_check=True)

            x_buf = xp.tile([128, PR, PW], wdt)

            def body():
                # ---- zero padding ----
                nc.gpsimd.memset(x_buf[:, :, 0:1], 0.0)
                nc.gpsimd.memset(x_buf[:, :, PW - 1:PW], 0.0)
                for g in range(NG):
                    if g % 2 == 0:
                        nc.gpsimd.memset(x_buf[32 * g:32 * g + 32, 0:1, 1:PW - 1], 0.0)
                    else:
                        nc.gpsimd.memset(x_buf[32 * g:32 * g + 32, PR - 1:PR, 1:PW - 1], 0.0)

                # ---- load x ----
                def ld(g, **kw):
                    if wdt != dt.float32:
                        nc.gpsimd.dma_start(**kw)
                    elif g % 2 == 0:
                        nc.sync.dma_start(**kw)
                    else:
                        nc.scalar.dma_start(**kw)
                for g in range(NG):
                    b, h = divmod(g, 2)
                    if h == 0:
                        ld(g, out=x_buf[32 * g:32 * g + 32, 1:65, 1:PW - 1],
                           in_=x_d[b, :, 0:64, :])
                    else:
                        ld(g, out=x_buf[32 * g:32 * g + 32, 0:65, 1:PW - 1],
                           in_=x_d[b, :, 127:192, :])
                for g in range(NG):
                    b, h = divmod(g, 2)
                    if h == 0:
                        ld(g, out=x_buf[32 * g:32 * g + 32, 65:130, 1:PW - 1],
                           in_=x_d[b, :, 64:129, :])
                    else:
                        ld(g, out=x_buf[32 * g:32 * g + 32, 65:129, 1:PW - 1],
                           in_=x_d[b, :, 192:256, :])

                # ---- per-partition sums (minus duplicated halo rows) ----
                sums = wk.tile([128, 4], dt.float32, name="sums", tag="sums")
                nc.vector.reduce_sum(sums[:, 0:1], x_buf[:, 0:65, :],
                                     axis=mybir.AxisListType.XY)
                nc.vector.reduce_sum(sums[:, 1:2], x_buf[:, 65:130, :],
                                     axis=mybir.AxisListType.XY)
                nc.vector.reduce_sum(sums[:, 2:3], x_buf[:, 0:1, :],
                                     axis=mybir.AxisListType.XY)
                nc.vector.reduce_sum(sums[:, 3:4], x_buf[:, PR - 1:PR, :],
                                     axis=mybir.AxisListType.XY)
                s_int = wk.tile([128, 1], dt.float32, name="s_int", tag="s_int")
                nc.vector.tensor_add(s_int[:], sums[:, 0:1], sums[:, 1:2])
                nc.vector.tensor_sub(s_int[:], s_int[:], sums[:, 2:3])
                nc.vector.tensor_sub(s_int[:], s_int[:], sums[:, 3:4])

                # ---- gating: one K=128 matmul folds combine + FC + /HW ----
                fc_ps = ps.tile([32, 1], dt.float32, name="fc_ps", tag="p_0_0")
                nc.tensor.matmul(fc_ps[:], fc2_sb[:], s_int[:], start=True, stop=True)
                sig = wk.tile([32, 1], dt.float32, name="sig", tag="sig")
                nc.scalar.activation(sig[:], fc_ps[:], AF.Sigmoid,
                                     bias=fcb2_sb[:], scale=1.0)

                # ---- broadcast (b,kind,e) -> (e,j) partitions via DRAM ----
                sig_dr = drp.tile([32, 1], dt.float32, name="sig_dr", tag="sig_dr")
                nc.sync.dma_start(out=sig_dr[:], in_=sig[:])
                ga_th = wk.tile([72, 4], dt.float32, name="ga_th", tag="ga_th")
                # sig row m = 16b + 8kind + e; dst col = 2b + kind -> addr 8*col+e
                for e in range(E):
                    nc.sync.dma_start(
                        out=ga_th[9 * e:9 * e + 9, :],
                        in_=bass.AP(tensor=sig_dr.tensor, offset=sig_dr.offset + e,
                                    ap=[[0, 9], [8, 4]]))
                th2 = ga_th.rearrange("p (b k) -> p b k", b=2)[:, :, 1]  # [72,2]
                trig = wk.tile([72, 4], dt.float32, name="trig", tag="trig")
                nc.scalar.activation(trig[:, 0:2], th2, AF.Sin,
                                     bias=halfpi_sb[:], scale=1.0)   # cos
                nc.scalar.activation(trig[:, 2:4], th2, AF.Sin,
                                     bias=0.0, scale=1.0)            # sin

                # ---- rotation matrices on (e,j) partitions ----
                A_sb = wk.tile([72, 18], dt.float32, name="A_sb", tag="A_sb")
                t1 = wk.tile([72, 9], dt.float32, name="t1", tag="t1")
                t2 = wk.tile([72, 9], dt.float32, name="t2", tag="t2")
                wr = wk.tile([72, 9], dt.float32, name="wr", tag="wr")
                for b in range(BL):
                    cos_b = trig[:, b:b + 1]
                    sin_b = trig[:, 2 + b:3 + b]
                    gsl = grt_sb[:, 9 * b:9 * b + 9]
                    csl = gct_sb[:, 9 * b:9 * b + 9]
                    # wr = relu(1 - |cos*Gr_t - sin*Gc_t - Gr_j|)
                    nc.vector.tensor_scalar_mul(t1[:], gsl, cos_b)
                    nc.vector.tensor_scalar_mul(t2[:], csl, sin_b)
                    nc.vector.tensor_sub(t1[:], t1[:], t2[:])
                    nc.vector.tensor_scalar_sub(t1[:], t1[:], grj_sb[:, 0:1])
                    nc.scalar.activation(t1[:], t1[:], AF.Abs)
                    nc.scalar.activation(wr[:], t1[:], AF.Relu, bias=1.0, scale=-1.0)
                    # wc = relu(1 - |sin*Gr_t + cos*Gc_t - Gc_j|)
                    nc.vector.tensor_scalar_mul(t1[:], gsl, sin_b)
                    nc.vector.tensor_scalar_mul(t2[:], csl, cos_b)
                    nc.vector.tensor_add(t1[:], t1[:], t2[:])
                    nc.vector.tensor_scalar_sub(t1[:], t1[:], gcj_sb[:, 0:1])
                    nc.scalar.activation(t1[:], t1[:], AF.Abs)
                    nc.scalar.activation(t1[:], t1[:], AF.Relu, bias=1.0, scale=-1.0)
                    nc.vector.tensor_mul(t1[:], t1[:], wr[:])
                    nc.vector.tensor_scalar_mul(A_sb[:, 9 * b:9 * b + 9],
                                                t1[:], ga_th[:, 2 * b:2 * b + 1])

                # ---- wout: 64 K=72 matmuls -> [32cin, (cout,t)] per image ----
                w_stage = wk.tile([32, 2 * 288], wdt, name="w_stage", tag="w_stage")
                for b in range(BL):
                    wps = ps.tile([32, 288], dt.float32, name=f"wps{b}",
                                  tag=f"p_{1 + b}_0")
                    for c in range(COUT):
                        nc.tensor.matmul(wps[:, 9 * c:9 * c + 9],
                                         wf_sb[:, 32 * c:32 * c + 32],
                                         A_sb[:, 9 * b:9 * b + 9],
                                         start=True, stop=True)
                    nc.vector.tensor_copy(w_stage[:, 288 * b:288 * b + 288], wps[:])

                # replicate to all 4 partition groups via DRAM
                w_dr = drp.tile([32, 2 * 288], wdt, name="w_dr", tag="w_dr")
                nc.sync.dma_start(out=w_dr[:], in_=w_stage[:])
                w_rep = wk.tile([128, 2 * 288], wdt, name="w_rep", tag="w_rep")
                for g in range(NG):
                    nc.sync.dma_start(out=w_rep[32 * g:32 * g + 32, :],
                                      in_=w_dr[:])
                w4 = w_rep.rearrange("p (b c t) -> p b c t", b=2, c=32, t=9)

                # ---- conv ----
                for s in range(ROUNDS):
                    P = [ps.tile([128, 512], dt.float32, name=f"P{g}_{s % 2}",
                                 tag=f"p_{g}_{s % 2}") for g in range(NG)]
                    for t in range(9):
                        dr_, dc_ = int(GRID_R[t]), int(GRID_C[t])
                        for g in range(NG):
                            b = g >> 1
                            lhsT = w4[32 * g:32 * g + 32, b, :, t]
                            for j in range(4):
                                r0 = 8 * s + 2 * j + dr_ + 1
                                rhs = x_buf[32 * g:32 * g + 32,
                                            r0:r0 + 2, 1 + dc_:257 + dc_]
                                nc.tensor.matmul(
                                    P[g][32 * j:32 * j + 32, :], lhsT, rhs,
                                    start=(t == 0), stop=(t == 8),
                                    tile_position=(32 * g, 32 * j))
                    for g in range(NG):
                        b, h = divmod(g, 2)
                        stg = stp.tile([128, 512], dt.float32, name=f"stg{g}",
                                       tag=f"stg{g % 2}")
                        if g % 2 == 0:
                            nc.vector.tensor_copy(stg[:], P[g][:])
                        else:
                            nc.scalar.copy(stg[:], P[g][:])
                        # dst dims ordered to match src linear order (j,c,r,w)
                        dst = bass.AP(
                            tensor=out_d, offset=b * COUT * H * W + (128 * h + 8 * s) * W,
                            ap=[[2 * W, 4], [H * W, 32], [1, 2 * W]])
                        (nc.sync if g % 2 == 0 else nc.scalar).dma_start(
                            out=dst, in_=stg[:])

            if loop:
                with tc.For_i(0, n_rv, 1):
                    body()
            else:
                body()
    nc.compile()
    return nc


def _host_prep(weight, fc_a_w, fc_a_b, fc_theta_w, fc_theta_b):
    wf_t = np.ascontiguousarray(
        weight.reshape(E, COUT, CIN, 9).transpose(0, 3, 1, 2).reshape(72, 1024)
    ).astype(np.float32)
    # fc2[(g,c), m=16b+8k+e] = fcw[k][e, c]/(H*W) if g>>1 == b else 0
    fc2 = np.zeros((128, 32), np.float32)
    for g in range(4):
        b = g >> 1
        for k, fw in enumerate((fc_a_w, fc_theta_w)):
            fc2[32 * g:32 * g + 32, 16 * b + 8 * k:16 * b + 8 * k + 8] = \
                fw.T.astype(np.float32) / (H * W)
    fcb2 = np.concatenate([fc_a_b, fc_theta_b] * 2).reshape(32, 1).astype(np.float32)
    grt = np.tile(GRID_R[None, :], (72, 2)).astype(np.float32)
    gct = np.tile(GRID_C[None, :], (72, 2)).astype(np.float32)
    grj = np.tile(GRID_R[None, :], (8, 1)).reshape(72, 1).astype(np.float32)
    gcj = np.tile(GRID_C[None, :], (8, 1)).reshape(72, 1).astype(np.float32)
    return dict(wf_t=wf_t, fc2=fc2, fcb2=fcb2, grt=grt, gct=gct, grj=grj, gcj=gcj)


_NC_CACHE = {}


def _get_module(wdt_name="f32", loop=False):
    key = (wdt_name, loop)
    if key not in _NC_CACHE:
        wdt = dt.float32 if wdt_name == "f32" else dt.bfloat16
        _NC_CACHE[key] = build_module(wdt, loop=loop)
    return _NC_CACHE[key]


def make_in_maps(x, weight, fc_a_w, fc_a_b, fc_theta_w, fc_theta_b, extra=None):
    small = _host_prep(weight, fc_a_w, fc_a_b, fc_theta_w, fc_theta_b)
    in_maps = []
    for i in range(NCORES):
        m = {"x": np.ascontiguousarray(x[BL * i:BL * i + BL])}
        m.update(small)
        if extra:
            m.update(extra)
        in_maps.append(m)
    return in_maps


def kernel(x, weight, fc_a_w, fc_a_b, fc_theta_w, fc_theta_b, _wdt="f32"):
    nc = _get_module(_wdt, loop=False)
    in_maps = make_in_maps(x, weight, fc_a_w, fc_a_b, fc_theta_w, fc_theta_b)
    res = run_bass_kernel_spmd(nc, in_maps, core_ids=list(range(NCORES)))
    return np.concatenate([r["out"] for r in res.results], axis=0)

